# revision 29
# baseline (speedup 1.0000x reference)
"""CenterLoss kernel for Trainium2 (Bass, raw engine programming), 8-core data-parallel.

Math: the reference builds the full (B, C) squared-distance matrix, masks it
to the true-label entry per row, clips to [1e-12, 1e12], sums, and divides by
B. Masked-out entries are exactly 0 before the clip, so each contributes
CLAMP_MIN after it. Hence

    loss = ( sum_i clip(||x_i - centers[labels_i]||^2, 1e-12, 1e12)
             + (B*C - B) * 1e-12 ) / B

which needs only a row gather + squared distance + reduction, not the
(B x C x D) matmul.

Distribution: batch rows are sharded across 8 cores (512 rows each); centers
stay in HBM on every core and each core gathers only the 512 rows it needs
via indirect DMA (one index per (partition, tile) slot). Each core returns
512 clipped per-row distances as a [128, 4] tile; the host does the final
tiny reduction.

Per-core dataflow (raw Bass; this toolchain's walrus rejects instructions
with more than one embedded semaphore wait, which rules out Tile, and cannot
encode the GPSIMD ucode-library ops, which rules out dma_gather):
  SP   : labels DMA -> x loads -> (after compute) result DMA out
  Pool : indirect-DMA center-row gathers once labels land
  DVE  : per 128-row tile: diff = x - c; last tile's square+reduce; clip
  ACT  : other tiles: acc[:, t] = row_sum(Square(diff))
Tile t holds global row t*128+p on partition p.
"""

from contextlib import ExitStack

import numpy as np

import concourse.bass as bass
import concourse.mybir as mybir
from concourse.bass_utils import run_bass_kernel_spmd

P = 128
B, C, D = 4096, 10000, 512
N_CORES = 8
ROWS = B // N_CORES   # 512 rows per core
NT = ROWS // P        # 4 tiles of 128 rows
NCHUNK = 2            # x-load chunks per core (gathers are always per-tile)
TPC = NT // NCHUNK    # tiles per chunk
CLAMP_MIN = 1e-12
CLAMP_MAX = 1e12

_cached_nc = None


def _build():
    nc = bass.Bass()
    x = nc.dram_tensor("x", [ROWS, D], mybir.dt.float32, kind="ExternalInput")
    # labels32[p, t] = labels[t*128 + p]
    lab32 = nc.dram_tensor("labels32", [P, NT], mybir.dt.int32, kind="ExternalInput")
    centers = nc.dram_tensor("centers", [C, D], mybir.dt.float32, kind="ExternalInput")
    out_d = nc.dram_tensor("out", [P, NT], mybir.dt.float32, kind="ExternalOutput")

    with ExitStack() as ctx:
        lab_t = ctx.enter_context(nc.sbuf_tensor("lab_t", [P, NT], mybir.dt.int32))
        xt = ctx.enter_context(nc.sbuf_tensor("xt", [P, NT, D], mybir.dt.float32))
        ct = ctx.enter_context(nc.sbuf_tensor("ct", [P, NT, D], mybir.dt.float32))
        diff = ctx.enter_context(nc.sbuf_tensor("diff", [P, NT, D], mybir.dt.float32))
        sq = ctx.enter_context(nc.sbuf_tensor("sq", [P, NT, D], mybir.dt.float32))
        acc = ctx.enter_context(nc.sbuf_tensor("acc", [P, NT], mybir.dt.float32))
        zero = ctx.enter_context(nc.sbuf_tensor("zero", [P, 1], mybir.dt.float32))
        scratch = ctx.enter_context(nc.sbuf_tensor("scratch", [P, 2], mybir.dt.float32))

        lab_sem = ctx.enter_context(nc.semaphore("lab_sem"))
        x_sems = [ctx.enter_context(nc.semaphore(f"x_sem{i}")) for i in range(NCHUNK)]
        c_sems = [ctx.enter_context(nc.semaphore(f"c_sem{i}")) for i in range(NT)]
        dve_sem = ctx.enter_context(nc.semaphore("dve_sem"))
        act_sem = ctx.enter_context(nc.semaphore("act_sem"))
        out_sem = ctx.enter_context(nc.semaphore("out_sem"))
        block = ctx.enter_context(nc.Block())

        rows_pc = ROWS // NCHUNK  # rows per chunk

        @block.sync
        def _(sync):
            sync.dma_start(out=lab_t[:], in_=lab32[:]).then_inc(lab_sem, 16)
            for i in range(NCHUNK):
                # xt[p, t, :] = x[t*128 + p, :] for chunk i's tiles t
                src = x[i * rows_pc:(i + 1) * rows_pc, :].rearrange(
                    "(j p) d -> p j d", j=TPC, p=P
                )
                sync.dma_start(
                    out=xt[:, i * TPC:(i + 1) * TPC, :], in_=src
                ).then_inc(x_sems[i], 16)
            sync.wait_ge(dve_sem, NT + 4)
            sync.dma_start(out=out_d[:], in_=acc[:]).then_inc(out_sem, 16)
            sync.wait_ge(out_sem, 16)

        @block.gpsimd
        def _(gpsimd):
            gpsimd.wait_ge(lab_sem, 16)
            # one gather per tile: the HW DGE only honors [P, 1] offset APs
            # (a [P, NT] offset AP gathers garbage on HW despite simulating
            # correctly), so feed it per-column views of the label tile.
            for t in range(NT):
                gpsimd.indirect_dma_start(
                    out=ct[:, t, :],
                    out_offset=None,
                    in_=centers[:],
                    in_offset=bass.IndirectOffsetOnAxis(
                        ap=lab_t[:, t:t + 1], axis=0
                    ),
                ).then_inc(c_sems[t], 16)

        @block.vector
        def _(vector):
            nc.vector.memset(zero[:], 0.0).then_inc(dve_sem, 1)
            for t in range(NT):
                if t % TPC == 0:
                    vector.wait_ge(x_sems[t // TPC], 16)
                vector.wait_ge(c_sems[t], 16)
                nc.vector.tensor_tensor(
                    out=diff[:, t, :], in0=xt[:, t, :], in1=ct[:, t, :],
                    op=mybir.AluOpType.subtract,
                ).then_inc(dve_sem, 1)
            # last tile's square+reduce on DVE to balance against ACT
            vector.wait_ge(dve_sem, NT + 1)
            nc.vector.tensor_tensor(
                out=sq[:, NT - 1, :], in0=diff[:, NT - 1, :], in1=diff[:, NT - 1, :],
                op=mybir.AluOpType.mult,
            ).then_inc(dve_sem, 1)
            vector.wait_ge(dve_sem, NT + 2)
            nc.vector.reduce_sum(
                out=acc[:, NT - 1:NT], in_=sq[:, NT - 1, :], axis=mybir.AxisListType.X,
            ).then_inc(dve_sem, 1)
            vector.wait_ge(dve_sem, NT + 3)
            vector.wait_ge(act_sem, NT)  # NT-1 real ops + 1 warmup
            # clip each per-row distance to [CLAMP_MIN, CLAMP_MAX]
            nc.vector.tensor_scalar(
                acc[:], acc[:], CLAMP_MIN, CLAMP_MAX,
                mybir.AluOpType.max, mybir.AluOpType.min,
            ).then_inc(dve_sem, 1)

        @block.scalar
        def _(scalar):
            # warm the ACT function table during the DMA window
            scalar.wait_ge(dve_sem, 1)  # zero tile ready
            nc.scalar.activation(
                out=scratch[:, 0:1],
                in_=zero[:, :1],
                func=mybir.ActivationFunctionType.Square,
                bias=zero[:, :1],
                scale=1.0,
                accum_out=scratch[:, 1:2],
            ).then_inc(act_sem, 1)
            for t in range(NT - 1):
                scalar.wait_ge(dve_sem, t + 2)  # memset + sub_t done
                nc.scalar.activation(
                    out=sq[:, t, :],
                    in_=diff[:, t, :],
                    func=mybir.ActivationFunctionType.Square,
                    bias=zero[:, :1],
                    scale=1.0,
                    accum_out=acc[:, t:t + 1],
                ).then_inc(act_sem, 1)

    return nc


def _prep_labels32(labels: np.ndarray) -> np.ndarray:
    """int32 [128, NT] with [p, t] = labels[t*128 + p]."""
    return np.ascontiguousarray(labels.astype(np.int32).reshape(NT, P).T)


def _run(inputs, trace=False):
    global _cached_nc
    if _cached_nc is None:
        _cached_nc = _build()
    nc = _cached_nc

    x = np.ascontiguousarray(np.asarray(inputs["x"], dtype=np.float32))
    labels = np.asarray(inputs["labels"])
    centers = np.ascontiguousarray(np.asarray(inputs["centers"], dtype=np.float32))

    in_maps = []
    for c in range(N_CORES):
        sl = slice(c * ROWS, (c + 1) * ROWS)
        in_maps.append({
            "x": x[sl],
            "labels32": _prep_labels32(labels[sl]),
            "centers": centers,
        })
    last_err = None
    for attempt in range(3):  # transient NRT exec errors recover on retry
        try:
            res = run_bass_kernel_spmd(nc, in_maps, list(range(N_CORES)), trace=trace)
            break
        except Exception as e:  # noqa: BLE001
            last_err = e
    else:
        raise last_err
    partials = np.stack([res.results[i]["out"] for i in range(N_CORES)])
    total = partials.astype(np.float64).sum()
    loss = total / B + (C - 1) * CLAMP_MIN
    return np.float32(loss), res


def kernel(**inputs) -> np.ndarray:
    val, _ = _run(inputs, trace=False)
    return np.asarray(val, dtype=np.float32)


# revision 31
# speedup vs baseline: 1.1467x; 1.1467x over previous
"""CenterLoss kernel for Trainium2 (Bass, raw engine programming), 8-core data-parallel.

Math: the reference builds the full (B, C) squared-distance matrix, masks it
to the true-label entry per row, clips to [1e-12, 1e12], sums, and divides by
B. Masked-out entries are exactly 0 before the clip, so each contributes
CLAMP_MIN after it. Hence

    loss = ( sum_i clip(||x_i - centers[labels_i]||^2, 1e-12, 1e12)
             + (B*C - B) * 1e-12 ) / B

which needs only a row gather + squared distance + reduction, not the
(B x C x D) matmul.

Distribution: batch rows are sharded across 8 cores (512 rows each); centers
stay in HBM on every core and each core gathers only the 512 rows it needs
via indirect DMA (one index per (partition, tile) slot). Each core returns
512 clipped per-row distances as a [128, 4] tile; the host does the final
tiny reduction.

Per-core dataflow (raw Bass; this toolchain's walrus rejects instructions
with more than one embedded semaphore wait, which rules out Tile, and cannot
encode the GPSIMD ucode-library ops, which rules out dma_gather):
  SP   : labels DMA -> x loads -> (after compute) result DMA out
  Pool : indirect-DMA center-row gathers once labels land
  DVE  : per 128-row tile: diff = x - c; last tile's square+reduce; clip
  ACT  : other tiles: acc[:, t] = row_sum(Square(diff))
Tile t holds global row t*128+p on partition p.
"""

from contextlib import ExitStack

import numpy as np

import concourse.bass as bass
import concourse.mybir as mybir
from concourse.bass_utils import run_bass_kernel_spmd

P = 128
B, C, D = 4096, 10000, 512
N_CORES = 8
ROWS = B // N_CORES   # 512 rows per core
NT = ROWS // P        # 4 tiles of 128 rows
NCHUNK = 4            # x-load chunks per core (gathers are always per-tile)
TPC = NT // NCHUNK    # tiles per chunk
CLAMP_MIN = 1e-12
CLAMP_MAX = 1e12

_cached_nc = None


def _build():
    nc = bass.Bass()
    x = nc.dram_tensor("x", [ROWS, D], mybir.dt.float32, kind="ExternalInput")
    # labels32[p, t] = labels[t*128 + p]
    lab32 = nc.dram_tensor("labels32", [P, NT], mybir.dt.int32, kind="ExternalInput")
    centers = nc.dram_tensor("centers", [C, D], mybir.dt.float32, kind="ExternalInput")
    out_d = nc.dram_tensor("out", [P, NT], mybir.dt.float32, kind="ExternalOutput")

    with ExitStack() as ctx:
        lab_t = ctx.enter_context(nc.sbuf_tensor("lab_t", [P, NT], mybir.dt.int32))
        xt = ctx.enter_context(nc.sbuf_tensor("xt", [P, NT, D], mybir.dt.float32))
        ct = ctx.enter_context(nc.sbuf_tensor("ct", [P, NT, D], mybir.dt.float32))
        diff = ctx.enter_context(nc.sbuf_tensor("diff", [P, NT, D], mybir.dt.float32))
        sq = ctx.enter_context(nc.sbuf_tensor("sq", [P, NT, D], mybir.dt.float32))
        acc = ctx.enter_context(nc.sbuf_tensor("acc", [P, NT], mybir.dt.float32))
        zero = ctx.enter_context(nc.sbuf_tensor("zero", [P, 1], mybir.dt.float32))
        scratch = ctx.enter_context(nc.sbuf_tensor("scratch", [P, 2], mybir.dt.float32))

        lab_sem = ctx.enter_context(nc.semaphore("lab_sem"))
        x_sems = [ctx.enter_context(nc.semaphore(f"x_sem{i}")) for i in range(NCHUNK)]
        c_sems = [ctx.enter_context(nc.semaphore(f"c_sem{i}")) for i in range(NT)]
        dve_sem = ctx.enter_context(nc.semaphore("dve_sem"))
        act_sem = ctx.enter_context(nc.semaphore("act_sem"))
        out_sem = ctx.enter_context(nc.semaphore("out_sem"))
        block = ctx.enter_context(nc.Block())

        rows_pc = ROWS // NCHUNK  # rows per chunk

        @block.sync
        def _(sync):
            for i in range(NCHUNK):
                # xt[p, t, :] = x[t*128 + p, :] for chunk i's tiles t
                src = x[i * rows_pc:(i + 1) * rows_pc, :].rearrange(
                    "(j p) d -> p j d", j=TPC, p=P
                )
                sync.dma_start(
                    out=xt[:, i * TPC:(i + 1) * TPC, :], in_=src
                ).then_inc(x_sems[i], 16)
            sync.wait_ge(dve_sem, NT + 4)
            sync.dma_start(out=out_d[:], in_=acc[:]).then_inc(out_sem, 16)
            sync.wait_ge(out_sem, 16)

        @block.gpsimd
        def _(gpsimd):
            # labels loaded by the Pool engine itself: the gathers observe the
            # completion without a cross-engine semaphore hop, which starts
            # descriptor generation ~500ns earlier than an SP-issued load.
            gpsimd.dma_start(out=lab_t[:], in_=lab32[:]).then_inc(lab_sem, 16)
            gpsimd.wait_ge(lab_sem, 16)
            # one gather per tile: the HW DGE only honors [P, 1] offset APs
            # (a [P, NT] offset AP gathers garbage on HW despite simulating
            # correctly), so feed it per-column views of the label tile.
            for t in range(NT):
                gpsimd.indirect_dma_start(
                    out=ct[:, t, :],
                    out_offset=None,
                    in_=centers[:],
                    in_offset=bass.IndirectOffsetOnAxis(
                        ap=lab_t[:, t:t + 1], axis=0
                    ),
                ).then_inc(c_sems[t], 16)

        @block.vector
        def _(vector):
            nc.vector.memset(zero[:], 0.0).then_inc(dve_sem, 1)
            for t in range(NT):
                if t % TPC == 0:
                    vector.wait_ge(x_sems[t // TPC], 16)
                vector.wait_ge(c_sems[t], 16)
                nc.vector.tensor_tensor(
                    out=diff[:, t, :], in0=xt[:, t, :], in1=ct[:, t, :],
                    op=mybir.AluOpType.subtract,
                ).then_inc(dve_sem, 1)
            # last tile's square+reduce on DVE to balance against ACT
            vector.wait_ge(dve_sem, NT + 1)
            nc.vector.tensor_tensor(
                out=sq[:, NT - 1, :], in0=diff[:, NT - 1, :], in1=diff[:, NT - 1, :],
                op=mybir.AluOpType.mult,
            ).then_inc(dve_sem, 1)
            vector.wait_ge(dve_sem, NT + 2)
            nc.vector.reduce_sum(
                out=acc[:, NT - 1:NT], in_=sq[:, NT - 1, :], axis=mybir.AxisListType.X,
            ).then_inc(dve_sem, 1)
            vector.wait_ge(dve_sem, NT + 3)
            vector.wait_ge(act_sem, NT)  # NT-1 real ops + 1 warmup
            # clip each per-row distance to [CLAMP_MIN, CLAMP_MAX]
            nc.vector.tensor_scalar(
                acc[:], acc[:], CLAMP_MIN, CLAMP_MAX,
                mybir.AluOpType.max, mybir.AluOpType.min,
            ).then_inc(dve_sem, 1)

        @block.scalar
        def _(scalar):
            # warm the ACT function table during the DMA window
            scalar.wait_ge(dve_sem, 1)  # zero tile ready
            nc.scalar.activation(
                out=scratch[:, 0:1],
                in_=zero[:, :1],
                func=mybir.ActivationFunctionType.Square,
                bias=zero[:, :1],
                scale=1.0,
                accum_out=scratch[:, 1:2],
            ).then_inc(act_sem, 1)
            for t in range(NT - 1):
                scalar.wait_ge(dve_sem, t + 2)  # memset + sub_t done
                nc.scalar.activation(
                    out=sq[:, t, :],
                    in_=diff[:, t, :],
                    func=mybir.ActivationFunctionType.Square,
                    bias=zero[:, :1],
                    scale=1.0,
                    accum_out=acc[:, t:t + 1],
                ).then_inc(act_sem, 1)

    return nc


def _prep_labels32(labels: np.ndarray) -> np.ndarray:
    """int32 [128, NT] with [p, t] = labels[t*128 + p]."""
    return np.ascontiguousarray(labels.astype(np.int32).reshape(NT, P).T)


def _run(inputs, trace=False):
    global _cached_nc
    if _cached_nc is None:
        _cached_nc = _build()
    nc = _cached_nc

    x = np.ascontiguousarray(np.asarray(inputs["x"], dtype=np.float32))
    labels = np.asarray(inputs["labels"])
    centers = np.ascontiguousarray(np.asarray(inputs["centers"], dtype=np.float32))

    in_maps = []
    for c in range(N_CORES):
        sl = slice(c * ROWS, (c + 1) * ROWS)
        in_maps.append({
            "x": x[sl],
            "labels32": _prep_labels32(labels[sl]),
            "centers": centers,
        })
    last_err = None
    for attempt in range(3):  # transient NRT exec errors recover on retry
        try:
            res = run_bass_kernel_spmd(nc, in_maps, list(range(N_CORES)), trace=trace)
            break
        except Exception as e:  # noqa: BLE001
            last_err = e
    else:
        raise last_err
    partials = np.stack([res.results[i]["out"] for i in range(N_CORES)])
    total = partials.astype(np.float64).sum()
    loss = total / B + (C - 1) * CLAMP_MIN
    return np.float32(loss), res


def kernel(**inputs) -> np.ndarray:
    val, _ = _run(inputs, trace=False)
    return np.asarray(val, dtype=np.float32)


# revision 33
# speedup vs baseline: 1.1826x; 1.0313x over previous
"""CenterLoss kernel for Trainium2 (Bass, raw engine programming), 8-core data-parallel.

Math: the reference builds the full (B, C) squared-distance matrix, masks it
to the true-label entry per row, clips to [1e-12, 1e12], sums, and divides by
B. Masked-out entries are exactly 0 before the clip, so each contributes
CLAMP_MIN after it. Hence

    loss = ( sum_i clip(||x_i - centers[labels_i]||^2, 1e-12, 1e12)
             + (B*C - B) * 1e-12 ) / B

which needs only a row gather + squared distance + reduction, not the
(B x C x D) matmul.

Distribution: batch rows are sharded across 8 cores (512 rows each); centers
stay in HBM on every core and each core gathers only the 512 rows it needs
via indirect DMA (one index per (partition, tile) slot). Each core returns
512 clipped per-row distances as a [128, 4] tile; the host does the final
tiny reduction.

Per-core dataflow (raw Bass; this toolchain's walrus rejects instructions
with more than one embedded semaphore wait, which rules out Tile, and cannot
encode the GPSIMD ucode-library ops, which rules out dma_gather):
  SP   : labels DMA -> x loads -> (after compute) result DMA out
  Pool : indirect-DMA center-row gathers once labels land
  DVE  : per 128-row tile: diff = x - c; last tile's square+reduce; clip
  ACT  : other tiles: acc[:, t] = row_sum(Square(diff))
Tile t holds global row t*128+p on partition p.
"""

from contextlib import ExitStack

import numpy as np

import concourse.bass as bass
import concourse.mybir as mybir
from concourse.bass_utils import run_bass_kernel_spmd

P = 128
B, C, D = 4096, 10000, 512
N_CORES = 8
ROWS = B // N_CORES   # 512 rows per core
NT = ROWS // P        # 4 tiles of 128 rows
NCHUNK = 4            # x-load chunks per core (gathers are always per-tile)
TPC = NT // NCHUNK    # tiles per chunk
CLAMP_MIN = 1e-12
CLAMP_MAX = 1e12

_cached_nc = None


def _build():
    nc = bass.Bass()
    x = nc.dram_tensor("x", [ROWS, D], mybir.dt.float32, kind="ExternalInput")
    # labels32[p, t] = labels[t*128 + p]
    lab32 = nc.dram_tensor("labels32", [P, NT], mybir.dt.int32, kind="ExternalInput")
    centers = nc.dram_tensor("centers", [C, D], mybir.dt.float32, kind="ExternalInput")
    out_d = nc.dram_tensor("out", [P, NT], mybir.dt.float32, kind="ExternalOutput")

    with ExitStack() as ctx:
        lab_t = ctx.enter_context(nc.sbuf_tensor("lab_t", [P, NT], mybir.dt.int32))
        xt = ctx.enter_context(nc.sbuf_tensor("xt", [P, NT, D], mybir.dt.float32))
        ct = ctx.enter_context(nc.sbuf_tensor("ct", [P, NT, D], mybir.dt.float32))
        diff = ctx.enter_context(nc.sbuf_tensor("diff", [P, NT, D], mybir.dt.float32))
        sq = ctx.enter_context(nc.sbuf_tensor("sq", [P, NT, D], mybir.dt.float32))
        junk = ctx.enter_context(nc.sbuf_tensor("junk", [P, D], mybir.dt.float32))
        acc = ctx.enter_context(nc.sbuf_tensor("acc", [P, NT], mybir.dt.float32))
        zero = ctx.enter_context(nc.sbuf_tensor("zero", [P, 1], mybir.dt.float32))
        scratch = ctx.enter_context(nc.sbuf_tensor("scratch", [P, 2], mybir.dt.float32))

        lab_sem = ctx.enter_context(nc.semaphore("lab_sem"))
        x_sems = [ctx.enter_context(nc.semaphore(f"x_sem{i}")) for i in range(NCHUNK)]
        c_sems = [ctx.enter_context(nc.semaphore(f"c_sem{i}")) for i in range(NT)]
        dve_sem = ctx.enter_context(nc.semaphore("dve_sem"))
        act_sem = ctx.enter_context(nc.semaphore("act_sem"))
        out_sem = ctx.enter_context(nc.semaphore("out_sem"))
        block = ctx.enter_context(nc.Block())

        rows_pc = ROWS // NCHUNK  # rows per chunk

        @block.sync
        def _(sync):
            for i in range(NCHUNK):
                # xt[p, t, :] = x[t*128 + p, :] for chunk i's tiles t
                src = x[i * rows_pc:(i + 1) * rows_pc, :].rearrange(
                    "(j p) d -> p j d", j=TPC, p=P
                )
                sync.dma_start(
                    out=xt[:, i * TPC:(i + 1) * TPC, :], in_=src
                ).then_inc(x_sems[i], 16)
            sync.wait_ge(dve_sem, NT + 4)
            sync.dma_start(out=out_d[:], in_=acc[:]).then_inc(out_sem, 16)
            sync.wait_ge(out_sem, 16)

        @block.gpsimd
        def _(gpsimd):
            # labels loaded by the Pool engine itself: the gathers observe the
            # completion without a cross-engine semaphore hop, which starts
            # descriptor generation ~500ns earlier than an SP-issued load.
            gpsimd.dma_start(out=lab_t[:], in_=lab32[:]).then_inc(lab_sem, 16)
            gpsimd.wait_ge(lab_sem, 16)
            # one gather per tile: the HW DGE only honors [P, 1] offset APs
            # (a [P, NT] offset AP gathers garbage on HW despite simulating
            # correctly), so feed it per-column views of the label tile.
            for t in range(NT):
                gpsimd.indirect_dma_start(
                    out=ct[:, t, :],
                    out_offset=None,
                    in_=centers[:],
                    in_offset=bass.IndirectOffsetOnAxis(
                        ap=lab_t[:, t:t + 1], axis=0
                    ),
                ).then_inc(c_sems[t], 16)

        @block.vector
        def _(vector):
            nc.vector.memset(zero[:], 0.0).then_inc(dve_sem, 1)
            for t in range(NT):
                if t % TPC == 0:
                    vector.wait_ge(x_sems[t // TPC], 16)
                vector.wait_ge(c_sems[t], 16)
                nc.vector.tensor_tensor(
                    out=diff[:, t, :], in0=xt[:, t, :], in1=ct[:, t, :],
                    op=mybir.AluOpType.subtract,
                ).then_inc(dve_sem, 1)
            # last tile's square+reduce on DVE to balance against ACT
            vector.wait_ge(dve_sem, NT + 1)
            nc.vector.tensor_tensor(
                out=sq[:, NT - 1, :], in0=diff[:, NT - 1, :], in1=diff[:, NT - 1, :],
                op=mybir.AluOpType.mult,
            ).then_inc(dve_sem, 1)
            vector.wait_ge(dve_sem, NT + 2)
            # row-sum via tensor_scalar(+0) with accum_out: fp32 tensor_scalar
            # runs in the DVE 2x_2p perf mode (both read ports on one input),
            # while InstTensorReduce is stuck at 1x — ~2x faster reduce.
            nc.vector.tensor_scalar(
                junk[:], sq[:, NT - 1, :], 0.0, None,
                mybir.AluOpType.add, mybir.AluOpType.add,
                acc[:, NT - 1:NT],
            ).then_inc(dve_sem, 1)
            vector.wait_ge(dve_sem, NT + 3)
            vector.wait_ge(act_sem, NT)  # NT-1 real ops + 1 warmup
            # clip each per-row distance to [CLAMP_MIN, CLAMP_MAX]
            nc.vector.tensor_scalar(
                acc[:], acc[:], CLAMP_MIN, CLAMP_MAX,
                mybir.AluOpType.max, mybir.AluOpType.min,
            ).then_inc(dve_sem, 1)

        @block.scalar
        def _(scalar):
            # warm the ACT function table during the DMA window
            scalar.wait_ge(dve_sem, 1)  # zero tile ready
            nc.scalar.activation(
                out=scratch[:, 0:1],
                in_=zero[:, :1],
                func=mybir.ActivationFunctionType.Square,
                bias=zero[:, :1],
                scale=1.0,
                accum_out=scratch[:, 1:2],
            ).then_inc(act_sem, 1)
            for t in range(NT - 1):
                scalar.wait_ge(dve_sem, t + 2)  # memset + sub_t done
                nc.scalar.activation(
                    out=sq[:, t, :],
                    in_=diff[:, t, :],
                    func=mybir.ActivationFunctionType.Square,
                    bias=zero[:, :1],
                    scale=1.0,
                    accum_out=acc[:, t:t + 1],
                ).then_inc(act_sem, 1)

    return nc


def _prep_labels32(labels: np.ndarray) -> np.ndarray:
    """int32 [128, NT] with [p, t] = labels[t*128 + p]."""
    return np.ascontiguousarray(labels.astype(np.int32).reshape(NT, P).T)


def _run(inputs, trace=False):
    global _cached_nc
    if _cached_nc is None:
        _cached_nc = _build()
    nc = _cached_nc

    x = np.ascontiguousarray(np.asarray(inputs["x"], dtype=np.float32))
    labels = np.asarray(inputs["labels"])
    centers = np.ascontiguousarray(np.asarray(inputs["centers"], dtype=np.float32))

    in_maps = []
    for c in range(N_CORES):
        sl = slice(c * ROWS, (c + 1) * ROWS)
        in_maps.append({
            "x": x[sl],
            "labels32": _prep_labels32(labels[sl]),
            "centers": centers,
        })
    last_err = None
    for attempt in range(3):  # transient NRT exec errors recover on retry
        try:
            res = run_bass_kernel_spmd(nc, in_maps, list(range(N_CORES)), trace=trace)
            break
        except Exception as e:  # noqa: BLE001
            last_err = e
    else:
        raise last_err
    partials = np.stack([res.results[i]["out"] for i in range(N_CORES)])
    total = partials.astype(np.float64).sum()
    loss = total / B + (C - 1) * CLAMP_MIN
    return np.float32(loss), res


def kernel(**inputs) -> np.ndarray:
    val, _ = _run(inputs, trace=False)
    return np.asarray(val, dtype=np.float32)
